# revision 29
# baseline (speedup 1.0000x reference)
"""GNN message-passing kernel for 8 TRN2 NeuronCores.

Strategy (sharded by destination node range, edges sorted by dst):
  reference per hop:
    messages = concat(h[src], h[dst]) @ W_msg + b_msg          [E, D]
    agg      = segment_sum(messages, dst)                      [N, D]
    h        = relu(concat(h, agg) @ W_upd + b_upd)            [N, D]
  Algebraic reduction (exact):
    agg = S @ Wm_top + (deg * h) @ Wm_bot + outer(deg, b_msg),
    where S = segment_sum(h[src], dst)  -- pure gather+segment-sum.
  Folding agg through the update GEMM (host-precomputed weight products):
    h_new = relu(h @ U_top + S @ A + (deg*h) @ B + outer(deg, c) + b_upd)
    A = Wm_top @ U_bot,  B = Wm_bot @ U_bot,  c = b_msg @ U_bot.
  This removes the O(E*D^2) edge GEMM entirely: per hop each core gathers
  h[src] rows for its ~E/8 edges (dma_gather round-robined over the 4 SWDGE
  queues so descriptor generation and DMA drain pipeline), segment-sums them
  with 0/1-indicator matmuls on the PE (edges sorted by dst, fixed pieces of
  128; the indicator matrices are generated on the Vector engine from
  dst-local ids instead of streaming them from HBM), then runs the fused
  node-update GEMM on its N/8 nodes.  Node states are exchanged once per hop
  with an AllGather of bf16 shards.
"""
import numpy as np
import ml_dtypes

import concourse.bacc as bacc
import concourse.mybir as mybir
import concourse.tile as tile
from concourse import bass_utils

# ---- problem constants (hardcoded per contract) ----
N = 25000
E = 400000
D = 256
F = 32
HOPS = 4
NTYPES = 10
NC_ = 8               # cores
PER = 3200            # dst rows per core (25600 padded)
NPAD = NC_ * PER      # 25600
TILES = PER // 128    # 25 dst tiles per core

BF16 = ml_dtypes.bfloat16
_nc_cache = {}


# ---------------- host-side preprocessing ----------------
def _prep(inputs):
    edges = np.asarray(inputs["edges"])
    src = edges[0].astype(np.int64)
    dst = edges[1].astype(np.int64)

    order = np.argsort(dst, kind="stable")
    src_s = src[order]
    dst_s = dst[order]

    # per (core, tile) edge lists
    tile_of = dst_s // 128          # global tile id 0..199
    counts = np.bincount(tile_of, minlength=NC_ * TILES)
    p_tile = int(np.ceil(counts.max() / 128))
    tile_e = p_tile * 128
    e_pad = TILES * tile_e

    starts = np.zeros(NC_ * TILES + 1, np.int64)
    np.cumsum(counts, out=starts[1:])

    idx_arrs, dloc_arrs, deg_arrs = [], [], []
    for c in range(NC_):
        idx_c = np.zeros(e_pad, np.int16)
        dl_c = np.full((TILES, 128, p_tile), 255.0, BF16)
        for t in range(TILES):
            g = c * TILES + t
            lo, hi = starts[g], starts[g + 1]
            n = hi - lo
            base = t * tile_e
            idx_c[base:base + n] = src_s[lo:hi].astype(np.int16)
            dloc = (dst_s[lo:hi] - (g * 128)).astype(np.int64)
            j = np.arange(n)
            dl_c[t, j % 128, j // 128] = dloc.astype(BF16)
        # wrap to [128, e_pad//16] int16, replicated across the 8 q7 stripes
        wrapped = idx_c.reshape(-1, 16).T
        idx_arr = np.zeros((128, e_pad // 16), np.int16)
        for k in range(8):
            idx_arr[16 * k:16 * (k + 1)] = wrapped
        idx_arrs.append(idx_arr)
        dloc_arrs.append(dl_c.transpose(1, 0, 2).reshape(128, TILES * p_tile))

        deg = np.bincount(dst_s, minlength=NPAD)[c * PER:(c + 1) * PER]
        deg_arrs.append(np.broadcast_to(deg.astype(np.float32), (128, PER)).astype(BF16))

    iota = np.broadcast_to((np.arange(tile_e) % 128).astype(BF16), (128, tile_e)).copy()

    # fused weights
    W_msg = np.asarray(inputs["W_msg"], np.float32)
    W_upd = np.asarray(inputs["W_upd"], np.float32)
    b_msg = np.asarray(inputs["b_msg"], np.float32)
    b_upd = np.asarray(inputs["b_upd"], np.float32)
    wf = np.zeros((HOPS, 6, 128, D), BF16)
    cvec = np.zeros((HOPS, 1, D), BF16)
    for i in range(HOPS):
        U_t = W_upd[i][:D]          # [256,256]
        U_b = W_upd[i][D:]
        A = W_msg[i][:D] @ U_b
        B = W_msg[i][D:] @ U_b
        for k in range(2):
            wf[i, 0 + k] = U_t[128 * k:128 * (k + 1)]
            wf[i, 2 + k] = A[128 * k:128 * (k + 1)]
            wf[i, 4 + k] = B[128 * k:128 * (k + 1)]
        cvec[i, 0] = b_msg[i] @ U_b
    b_upd_t = np.zeros((128, 2 * HOPS), np.float32)
    for i in range(HOPS):
        b_upd_t[:, 2 * i] = b_upd[i][:128]
        b_upd_t[:, 2 * i + 1] = b_upd[i][128:]

    # h0 ingredients (per-core local shard)
    nodes = np.asarray(inputs["nodes"], np.float32)
    node_types = np.asarray(inputs["node_types"], np.int64)
    type_emb_eff = np.asarray(inputs["type_emb"], np.float32) + np.asarray(inputs["b_proj"], np.float32)[None, :]
    te = np.zeros((16, D), BF16)
    te[:NTYPES] = type_emb_eff.astype(BF16)
    W_proj = np.asarray(inputs["W_proj"], np.float32).astype(BF16)  # [32,256]

    nodes_T, onehot, vmask = [], [], []
    for c in range(NC_):
        nt = np.zeros((F, PER), BF16)
        oh = np.zeros((16, PER), BF16)
        vm = np.zeros((128, PER), BF16)
        lo = c * PER
        hi = min(N, lo + PER)
        nv = hi - lo
        if nv > 0:
            nt[:, :nv] = nodes[lo:hi].T.astype(BF16)
            oh[node_types[lo:hi], np.arange(nv)] = 1.0
            vm[:, :nv] = 1.0
        nodes_T.append(nt)
        onehot.append(oh)
        vmask.append(vm)

    ident = np.eye(128, dtype=BF16)
    per_core = []
    for c in range(NC_):
        per_core.append(dict(
            idx=idx_arrs[c], dloc=dloc_arrs[c], iota=iota, degb=deg_arrs[c],
            nodesT=nodes_T[c], onehot=onehot[c], vmask=vmask[c],
            wf=wf, cvec=cvec, bupd=b_upd_t, te=te, wproj=W_proj, ident=ident,
        ))
    return per_core, p_tile


# ---------------- device kernel ----------------
def _build(p_tile):
    tile_e = p_tile * 128
    e_pad = TILES * tile_e
    fp32 = mybir.dt.float32
    bf16 = mybir.dt.bfloat16

    nc = bacc.Bacc("TRN2", target_bir_lowering=False, debug=False,
                   enable_asserts=True, num_devices=NC_,
                   num_swdge_queues=4)
    # inputs
    idx_d = nc.dram_tensor("idx", [128, e_pad // 16], mybir.dt.int16, kind="ExternalInput")
    dloc_d = nc.dram_tensor("dloc", [128, TILES * p_tile], bf16, kind="ExternalInput")
    iota_d = nc.dram_tensor("iota", [128, tile_e], bf16, kind="ExternalInput")
    degb_d = nc.dram_tensor("degb", [128, PER], bf16, kind="ExternalInput")
    nodesT_d = nc.dram_tensor("nodesT", [F, PER], bf16, kind="ExternalInput")
    onehot_d = nc.dram_tensor("onehot", [16, PER], bf16, kind="ExternalInput")
    vmask_d = nc.dram_tensor("vmask", [128, PER], bf16, kind="ExternalInput")
    wf_d = nc.dram_tensor("wf", [HOPS, 6, 128, D], bf16, kind="ExternalInput")
    cvec_d = nc.dram_tensor("cvec", [HOPS, 1, D], bf16, kind="ExternalInput")
    bupd_d = nc.dram_tensor("bupd", [128, 2 * HOPS], fp32, kind="ExternalInput")
    te_d = nc.dram_tensor("te", [16, D], bf16, kind="ExternalInput")
    wproj_d = nc.dram_tensor("wproj", [F, D], bf16, kind="ExternalInput")
    ident_d = nc.dram_tensor("ident", [128, 128], bf16, kind="ExternalInput")
    # output
    locmax_d = nc.dram_tensor("locmax", [128, 2], fp32, kind="ExternalOutput")

    RELU = mybir.ActivationFunctionType.Relu
    COPY = mybir.ActivationFunctionType.Copy
    EQ = mybir.AluOpType.is_equal

    with tile.TileContext(nc) as tc:
        with (
            tc.tile_pool(name="dram", bufs=1, space="DRAM") as dram,
            tc.tile_pool(name="stat", bufs=1) as stat,
            tc.tile_pool(name="mstream", bufs=4) as mpool,
            tc.tile_pool(name="gpool", bufs=3) as gpool,
            tc.tile_pool(name="hT", bufs=2) as hTpool,
            tc.tile_pool(name="work", bufs=1) as work,
            tc.tile_pool(name="spsum", bufs=2, space="PSUM") as spsum,
            tc.tile_pool(name="tpsum", bufs=2, space="PSUM") as tpsum,
            tc.tile_pool(name="upsum", bufs=2, space="PSUM") as upsum,
        ):
            # static SBUF loads
            idx_t = stat.tile([128, e_pad // 16], mybir.dt.int16)
            nc.sync.dma_start(idx_t[:], idx_d[:])
            dloc_sb = stat.tile([128, TILES * p_tile], bf16)
            nc.sync.dma_start(dloc_sb[:], dloc_d[:])
            iota_sb = stat.tile([128, tile_e], bf16)
            nc.sync.dma_start(iota_sb[:], iota_d[:])
            degb = stat.tile([128, PER], bf16)
            nc.sync.dma_start(degb[:], degb_d[:])
            vmask = stat.tile([128, PER], bf16)
            nc.sync.dma_start(vmask[:], vmask_d[:])
            wf_sb = stat.tile([128, HOPS * 6 * D], bf16, name="wf_sb")
            nc.sync.dma_start(
                wf_sb[:].rearrange("p (h s d) -> p h s d", h=HOPS, s=6),
                wf_d.rearrange("h s p d -> p h s d"),
            )
            cvec_sb = stat.tile([1, HOPS * D], bf16)
            nc.sync.dma_start(
                cvec_sb[:].rearrange("o (h d) -> o h d", h=HOPS),
                cvec_d.rearrange("h o d -> o h d"),
            )
            bupd_sb = stat.tile([128, 2 * HOPS], fp32)
            nc.sync.dma_start(bupd_sb[:], bupd_d[:])
            te_sb = stat.tile([16, D], bf16)
            nc.sync.dma_start(te_sb[:], te_d[:])
            wproj_sb = stat.tile([F, D], bf16)
            nc.sync.dma_start(wproj_sb[:], wproj_d[:])
            ident = stat.tile([128, 128], bf16)
            nc.sync.dma_start(ident[:], ident_d[:])
            nodesT_sb = stat.tile([F, PER], bf16)
            nc.sync.dma_start(nodesT_sb[:], nodesT_d[:])
            onehot_sb = stat.tile([16, PER], bf16)
            nc.sync.dma_start(onehot_sb[:], onehot_d[:])

            # DRAM tables
            tabs = []
            for i in range(HOPS):
                tabs.append(dram.tile([NPAD, D], bf16, addr_space="Shared",
                                      name=f"htab{i}", tag=f"htab{i}"))
            bounces = [dram.tile([PER, D], bf16, name=f"bnc{i}", tag=f"bnc{i}")
                       for i in range(HOPS)]

            def wf_chunk(hop, s, fo):
                # lhsT [128 fi, 128 fo] slice of fused weight chunk s for hop
                base = (hop * 6 + s) * D
                return wf_sb[:, base + fo * 128: base + (fo + 1) * 128]

            def make_m(t):
                # indicator matrix for tile t: m[e, p*128+d] = (dloc[e,p]==d)
                m_t = mpool.tile([128, tile_e], bf16, name="mt", tag="mt")
                nc.vector.tensor_tensor(
                    m_t[:].rearrange("q (p e) -> q p e", e=128),
                    iota_sb[:].rearrange("q (p e) -> q p e", e=128),
                    dloc_sb[:, t * p_tile:(t + 1) * p_tile]
                        .unsqueeze(2).broadcast_to([128, p_tile, 128]),
                    op=EQ)
                return m_t

            def rowmajor_store_and_T(src_T0, src_T1, row_stage):
                for t in range(TILES):
                    for fo in range(2):
                        srcT = src_T0 if fo == 0 else src_T1
                        tp = tpsum.tile([128, 128], bf16, name="tp", tag="tp")
                        nc.tensor.transpose(
                            tp[:], srcT[:, t * 128:(t + 1) * 128], ident[:])
                        nc.vector.tensor_copy(
                            row_stage[:, t * 256 + fo * 128: t * 256 + (fo + 1) * 128],
                            tp[:])

            # ---- h0 phase: local shard only ----
            hT0 = hTpool.tile([128, PER], bf16, name="hTa", tag="hTa")
            hT1 = hTpool.tile([128, PER], bf16, name="hTb", tag="hTb")
            row_stage = work.tile([128, TILES * 256], bf16, name="rowstage")
            for t in range(TILES):
                ps = upsum.tile([128, 512], fp32, name="ups", tag="ups")
                nc.tensor.matmul(ps[:, :D], nodesT_sb[:, t * 128:(t + 1) * 128],
                                 wproj_sb[:], start=True, stop=False)
                nc.tensor.matmul(ps[:, :D], onehot_sb[:, t * 128:(t + 1) * 128],
                                 te_sb[:], start=False, stop=True)
                nc.scalar.activation(row_stage[:, t * 256:(t + 1) * 256], ps[:, :D], COPY)
                for fo in range(2):
                    tp = tpsum.tile([128, 128], bf16, name="tp", tag="tp")
                    nc.tensor.transpose(tp[:], row_stage[:, t * 256 + fo * 128:t * 256 + (fo + 1) * 128], ident[:])
                    dst = hT0 if fo == 0 else hT1
                    nc.vector.tensor_copy(dst[:, t * 128:(t + 1) * 128], tp[:])
            nc.sync.dma_start(
                bounces[0].rearrange("(t p) f -> p t f", p=128),
                row_stage[:].rearrange("p (t f) -> p t f", f=256))
            nc.gpsimd.collective_compute(
                "AllGather", mybir.AluOpType.bypass,
                replica_groups=[list(range(NC_))],
                ins=[bounces[0].opt()], outs=[tabs[0].opt()],
            )

            hT = [hT0, hT1]
            # ---- hops ----
            for i in range(HOPS):
                S_T0 = work.tile([128, PER], bf16, name="st0", tag="st0")
                S_T1 = work.tile([128, PER], bf16, name="st1", tag="st1")
                hdeg0 = work.tile([128, PER], bf16, name="hd0", tag="hd0")
                hdeg1 = work.tile([128, PER], bf16, name="hd1", tag="hd1")
                nc.vector.tensor_tensor(hdeg0[:], hT[0][:], degb[:], op=mybir.AluOpType.mult)
                nc.vector.tensor_tensor(hdeg1[:], hT[1][:], degb[:], op=mybir.AluOpType.mult)
                hTn0 = hTpool.tile([128, PER], bf16, name="hTa", tag="hTa")
                hTn1 = hTpool.tile([128, PER], bf16, name="hTb", tag="hTb")
                states = [hT[0], hT[1], S_T0, S_T1, hdeg0, hdeg1]
                last = i == HOPS - 1
                if not last:
                    row_stage2 = work.tile([128, TILES * 256], bf16, name="rowstage")

                for b0 in range(0, PER, 512):
                    bs = min(512, PER - b0)
                    for t in range(b0 // 128, (b0 + bs) // 128):
                        if t % 2 == 0:
                            # paired gather: tiles t, t+1 in one call to
                            # amortize the ~3.7us fixed Q7 cost per call
                            nt = 2 if t + 1 < TILES else 1
                            g = gpool.tile([128, 2 * p_tile * D], bf16,
                                           name="g", tag="g")
                            n = nt * tile_e
                            nc.gpsimd.dma_gather(
                                g[:, :nt * p_tile * D].rearrange(
                                    "q (p d) -> q p d", d=D),
                                tabs[i][:, :],
                                idx_t[:, t * (tile_e // 16):
                                      t * (tile_e // 16) + n // 16],
                                num_idxs=n, num_idxs_reg=n, elem_size=D,
                                single_packet=False, queue_num=(t // 2) % 4,
                            )
                        off = (t % 2) * p_tile
                        m_t = make_m(t)
                        sp = spsum.tile([128, D], fp32, name="sp", tag="sp")
                        for p in range(p_tile):
                            nc.tensor.matmul(sp[:], m_t[:, p * 128:(p + 1) * 128],
                                             g[:, (off + p) * D:(off + p + 1) * D],
                                             start=(p == 0),
                                             stop=(p == p_tile - 1))
                        s_sb = work.tile([128, D], bf16, name="ssb", tag="ssb", bufs=2)
                        nc.scalar.activation(s_sb[:], sp[:], COPY)
                        for fo in range(2):
                            tp = tpsum.tile([128, 128], bf16, name="tp", tag="tp")
                            nc.tensor.transpose(tp[:], s_sb[:, fo * 128:(fo + 1) * 128], ident[:])
                            dst = S_T0 if fo == 0 else S_T1
                            nc.vector.tensor_copy(dst[:, t * 128:(t + 1) * 128], tp[:])

                    for fo in range(2):
                        hTn = hTn0 if fo == 0 else hTn1
                        ps = upsum.tile([128, 512], fp32, name="ups", tag="ups")
                        for s in range(6):
                            nc.tensor.matmul(ps[:, :bs], wf_chunk(i, s, fo),
                                             states[s][:, b0:b0 + bs],
                                             start=(s == 0), stop=False)
                        nc.tensor.matmul(ps[:, :bs],
                                         cvec_sb[:, i * D + fo * 128: i * D + (fo + 1) * 128],
                                         degb[0:1, b0:b0 + bs],
                                         start=False, stop=True)
                        nc.scalar.activation(hTn[:, b0:b0 + bs], ps[:, :bs], RELU,
                                             bias=bupd_sb[:, 2 * i + fo: 2 * i + fo + 1])
                    # mask + row-major transposes for the finished block
                    nc.vector.tensor_tensor(hTn0[:, b0:b0 + bs], hTn0[:, b0:b0 + bs],
                                            vmask[:, b0:b0 + bs], op=mybir.AluOpType.mult)
                    nc.vector.tensor_tensor(hTn1[:, b0:b0 + bs], hTn1[:, b0:b0 + bs],
                                            vmask[:, b0:b0 + bs], op=mybir.AluOpType.mult)
                    if not last:
                        for t in range(b0 // 128, (b0 + bs) // 128):
                            for fo in range(2):
                                srcT = hTn0 if fo == 0 else hTn1
                                tp = tpsum.tile([128, 128], bf16, name="tp", tag="tp")
                                nc.tensor.transpose(
                                    tp[:], srcT[:, t * 128:(t + 1) * 128], ident[:])
                                nc.vector.tensor_copy(
                                    row_stage2[:, t * 256 + fo * 128: t * 256 + (fo + 1) * 128],
                                    tp[:])
                hT = [hTn0, hTn1]

                if not last:
                    nc.sync.dma_start(
                        bounces[i + 1].rearrange("(t p) f -> p t f", p=128),
                        row_stage2[:].rearrange("p (t f) -> p t f", f=256))
                    nc.gpsimd.collective_compute(
                        "AllGather", mybir.AluOpType.bypass,
                        replica_groups=[list(range(NC_))],
                        ins=[bounces[i + 1].opt()], outs=[tabs[i + 1].opt()],
                    )

            # ---- final local max ----
            lm = stat.tile([128, 2], fp32)
            nc.vector.reduce_max(lm[:, 0:1], hT[0][:], axis=mybir.AxisListType.X)
            nc.vector.reduce_max(lm[:, 1:2], hT[1][:], axis=mybir.AxisListType.X)
            nc.sync.dma_start(locmax_d[:], lm[:])

    nc.compile()
    return nc


def kernel(**inputs) -> np.ndarray:
    per_core, p_tile = _prep(inputs)
    if p_tile not in _nc_cache:
        _nc_cache[p_tile] = _build(p_tile)
    nc = _nc_cache[p_tile]
    in_maps = [
        dict(idx=pc["idx"], dloc=pc["dloc"], iota=pc["iota"], degb=pc["degb"],
             nodesT=pc["nodesT"], onehot=pc["onehot"], vmask=pc["vmask"],
             wf=pc["wf"], cvec=pc["cvec"], bupd=pc["bupd"], te=pc["te"],
             wproj=pc["wproj"], ident=pc["ident"])
        for pc in per_core
    ]
    res = bass_utils.run_bass_kernel_spmd(nc, in_maps, list(range(NC_)), trace=False)
    lm = np.stack([res.results[c]["locmax"] for c in range(NC_)])  # [8,128,2]
    gmax = lm.max(axis=0).T.reshape(D)  # feat fo*128+p
    W_out = np.asarray(inputs["W_out"], np.float32)
    b_out = np.asarray(inputs["b_out"], np.float32)
    return (gmax @ W_out + b_out).astype(np.float32)
